# revision 6
# baseline (speedup 1.0000x reference)
"""AdaptiveInput embedding lookup kernel for TRN2 (8 NeuronCores).

Strategy: pure data-parallel over tokens — one batch row (4096 tokens) per
core, tables replicated, no collectives.

Each core's tokens are partitioned by cluster HOST-side into three compacted
index lists (sorted by token id within a cluster for HBM locality), padded to
fixed capacities:
  head  (t < 5000):           gather f32 1024-wide rows -> scatter, no compute
  tail0 (5000 <= t < 20000):  gather bf16 512-wide rows TRANSPOSED
  tail1 (20000 <= t):         gather bf16 256-wide rows TRANSPOSED

Gathers use the SWDGE dma_gather instruction, chunked at <=512 indices per
op (single ops with >=1024 descriptors crash the device — SWDGE ring
capacity).  transpose=True delivers the embedding rows with the contraction
dim on partitions (eT layout), so the tail matmuls against pre-transposed
bf16 weights need NO PE transposes, no PSUM round-trip for lhsT, and no
masking — every slot of 128 compacted tokens is homogeneous.  Projected
slots are copied PSUM->SBUF (alternating ACT/DVE) and scattered to the
token's original output row with per-slot [128,1]-offset indirect DMAs (the
only HW-validated indirect-scatter shape); padding entries carry an
out-of-bounds destination and are silently dropped (bounds_check,
oob_is_err=False).
"""

import numpy as np

import concourse.bass as bass
import concourse.mybir as mybir
import concourse.tile as tile
from concourse import bacc
from concourse.bass import IndirectOffsetOnAxis

FP32 = mybir.dt.float32
BF16 = mybir.dt.bfloat16
I32 = mybir.dt.int32
I16 = mybir.dt.int16

P = 128
D = 1024  # IN_FEATURES
HEAD_ROWS = 5000
T0_ROWS, T0_H = 15000, 512
T1_ROWS, T1_H = 30257, 256
CUT1, CUT2 = 5000, 20000
N_TOK = 4096

# fixed per-core cluster capacities (multiples of 128); observed per-core
# counts on the reference inputs: head <= 445, tail0 <= 1260, tail1 <= 2492
CAP0 = 512
CAP1 = 1408
CAP2 = 2816
S0 = CAP0 // P   # 4 head slots
S1 = CAP1 // P   # 11 tail0 slots
S2 = CAP2 // P   # 22 tail1 slots
PAD_DEST = 1 << 20

N_CORES = 8


def build_graph():
    nk0 = T0_H // P  # 4
    nk1 = T1_H // P  # 2

    nc = bacc.Bacc("TRN2", target_bir_lowering=False, debug=False)

    head_tab = nc.dram_tensor("head_tab", [HEAD_ROWS, D], FP32,
                              kind="ExternalInput")
    t0_tab = nc.dram_tensor("t0_tab", [T0_ROWS, T0_H], BF16,
                            kind="ExternalInput")
    t1_tab = nc.dram_tensor("t1_tab", [T1_ROWS, T1_H], BF16,
                            kind="ExternalInput")
    w0T_ext = nc.dram_tensor("w0T", [P, nk0, D], BF16, kind="ExternalInput")
    w1T_ext = nc.dram_tensor("w1T", [P, nk1, D], BF16, kind="ExternalInput")
    g0_ext = nc.dram_tensor("g0", [P, CAP0 // 16], I16, kind="ExternalInput")
    g1_ext = nc.dram_tensor("g1", [P, CAP1 // 16], I16, kind="ExternalInput")
    g2_ext = nc.dram_tensor("g2", [P, CAP2 // 16], I16, kind="ExternalInput")
    d0_ext = nc.dram_tensor("d0", [P, S0], I32, kind="ExternalInput")
    d1_ext = nc.dram_tensor("d1", [P, S1], I32, kind="ExternalInput")
    d2_ext = nc.dram_tensor("d2", [P, S2], I32, kind="ExternalInput")
    out_ext = nc.dram_tensor("out", [N_TOK, D], FP32, kind="ExternalOutput")

    with tile.TileContext(nc) as tc:
        with (
            tc.tile_pool(name="const", bufs=1) as cpool,
            tc.tile_pool(name="wout", bufs=6) as wpool,
            tc.tile_pool(name="ps", bufs=4, space="PSUM") as pspool,
        ):
            v = nc.vector

            # ---- one-time loads: gather indices, scatter offsets, weights
            g0 = cpool.tile([P, CAP0 // 16], I16, tag="g0")
            g1 = cpool.tile([P, CAP1 // 16], I16, tag="g1")
            g2 = cpool.tile([P, CAP2 // 16], I16, tag="g2")
            d0 = cpool.tile([P, S0], I32, tag="d0")
            d1 = cpool.tile([P, S1], I32, tag="d1")
            d2 = cpool.tile([P, S2], I32, tag="d2")
            nc.sync.dma_start(g0[:], g0_ext[:, :])
            nc.sync.dma_start(g1[:], g1_ext[:, :])
            nc.sync.dma_start(g2[:], g2_ext[:, :])
            nc.sync.dma_start(d0[:], d0_ext[:, :])
            nc.sync.dma_start(d1[:], d1_ext[:, :])
            nc.sync.dma_start(d2[:], d2_ext[:, :])

            w0T = cpool.tile([P, nk0, D], BF16, tag="w0T")
            w1T = cpool.tile([P, nk1, D], BF16, tag="w1T")
            nc.sync.dma_start(w0T[:], w0T_ext[:, :, :])
            nc.sync.dma_start(w1T[:], w1T_ext[:, :, :])

            # HAM warm-up: dependency-free matmuls so the PE clock reaches
            # 8/8 before the first real matmuls arrive
            warm = cpool.tile([P, 512], BF16, tag="warm")
            nc.vector.memset(warm[:], 0.0)
            wps = pspool.tile([P, D], FP32, tag="mm", name="warmps")
            for _ in range(16):
                nc.tensor.matmul(out=wps[:, 0:512], lhsT=warm[:, 0:P],
                                 rhs=warm[:], start=True, stop=True)

            # ---- gathers: chunked SWDGE ops (<=512 idxs each) ----
            def chunk_sizes(cap):
                out = []
                left = cap
                while left > 0:
                    c = min(512, left)
                    out.append(c)
                    left -= c
                return out

            H = cpool.tile([P, S0, D], FP32, tag="H")
            nc.gpsimd.dma_gather(H[:], head_tab[:, :], g0[:], CAP0, CAP0, D)

            # (chunk_tile, tokens_in_chunk) lists; chunk c covers compact
            # tokens [tok0, tok0+n) of its cluster
            eT0c, eT1c = [], []
            tok0 = 0
            for ci, n in enumerate(chunk_sizes(CAP1)):
                e = cpool.tile([P, nk0, n], BF16, tag=f"eT0_{ci}")
                nc.gpsimd.dma_gather(
                    e[:], t0_tab[:, :], g1[:, tok0 // 16:(tok0 + n) // 16],
                    n, n, T0_H, transpose=True)
                eT0c.append((e, n))
                tok0 += n
            tok0 = 0
            for ci, n in enumerate(chunk_sizes(CAP2)):
                e = cpool.tile([P, nk1, n], BF16, tag=f"eT1_{ci}")
                nc.gpsimd.dma_gather(
                    e[:], t1_tab[:, :], g2[:, tok0 // 16:(tok0 + n) // 16],
                    n, n, T1_H, transpose=True)
                eT1c.append((e, n))
                tok0 += n

            # head: scatter straight from the gathered tile, no compute
            for s in range(S0):
                nc.gpsimd.indirect_dma_start(
                    out=out_ext[:, :],
                    out_offset=IndirectOffsetOnAxis(ap=d0[:, s:s + 1], axis=0),
                    in_=H[:, s, :], in_offset=None,
                    bounds_check=N_TOK - 1, oob_is_err=False)

            # ---- tails: per-slot matmul -> PSUM -> SBUF -> scatter ----
            slot_ctr = 0

            def do_slot(eT, sc, wT, nk, d_tile, gs, label):
                """sc: slot within chunk tile eT; gs: global slot index"""
                nonlocal slot_ctr
                mm = pspool.tile([P, D], FP32, tag="mm",
                                 name=f"mm_{label}_{gs}")
                for h in range(2):
                    fs = slice(h * 512, (h + 1) * 512)
                    for kc in range(nk):
                        nc.tensor.matmul(
                            out=mm[:, fs],
                            lhsT=eT[:, kc, sc * P:(sc + 1) * P],
                            rhs=wT[:, kc, fs],
                            start=(kc == 0), stop=(kc == nk - 1))
                W = wpool.tile([P, D], FP32, tag="W", name=f"W_{label}_{gs}")
                if slot_ctr % 2 == 0:
                    nc.scalar.copy(out=W[:], in_=mm[:])
                else:
                    v.tensor_copy(W[:], mm[:])
                slot_ctr += 1
                nc.gpsimd.indirect_dma_start(
                    out=out_ext[:, :],
                    out_offset=IndirectOffsetOnAxis(ap=d_tile[:, gs:gs + 1],
                                                    axis=0),
                    in_=W[:], in_offset=None,
                    bounds_check=N_TOK - 1, oob_is_err=False)

            gs = 0
            for e, n in eT0c:
                for sc in range(n // P):
                    do_slot(e, sc, w0T, nk0, d1, gs, "t0")
                    gs += 1
            gs = 0
            for e, n in eT1c:
                for sc in range(n // P):
                    do_slot(e, sc, w1T, nk1, d2, gs, "t1")
                    gs += 1

    nc.compile()
    return nc


_GRAPH_CACHE = {}


def _get_graph():
    if "g" not in _GRAPH_CACHE:
        _GRAPH_CACHE["g"] = build_graph()
    return _GRAPH_CACHE["g"]


def make_wT(w, h):
    """[D, h] f32 -> [128, h//128, D] bf16 with (p, kc, f) = w[f, kc*128+p]"""
    import ml_dtypes

    wt = np.ascontiguousarray(w, dtype=np.float32).T  # [h, D]
    wt = wt.reshape(h // P, P, D).transpose(1, 0, 2)  # [P, h//128, D]
    return np.ascontiguousarray(wt.astype(ml_dtypes.bfloat16))


def _wrap16(a):
    """[cap] -> [128, cap//16] int16: idx i at (p = i%16 (replicated x8),
    col = i//16) — the SWDGE gather index layout."""
    m = a.reshape(-1, 16).T  # [16, cap//16]
    return np.ascontiguousarray(np.tile(m, (8, 1)))


def _slots(a):
    """[cap] -> [128, cap//128] int32: compact token i at (p = i%128,
    slot = i//128) — the scatter offset layout."""
    return np.ascontiguousarray(a.reshape(-1, P).T)


def make_in_maps(tokens, head_emb, tail0_emb, tail0_w, tail1_emb, tail1_w):
    import ml_dtypes

    head = np.ascontiguousarray(head_emb, dtype=np.float32)
    t0b = np.ascontiguousarray(
        np.asarray(tail0_emb, dtype=np.float32).astype(ml_dtypes.bfloat16))
    t1b = np.ascontiguousarray(
        np.asarray(tail1_emb, dtype=np.float32).astype(ml_dtypes.bfloat16))
    w0T = make_wT(tail0_w, T0_H)
    w1T = make_wT(tail1_w, T1_H)

    maps = []
    for b in range(tokens.shape[0]):
        t = np.asarray(tokens[b], dtype=np.int64).reshape(-1)
        cl = (t >= CUT1).astype(np.int8) + (t >= CUT2).astype(np.int8)
        gs, ds = [], []
        for ci, (lo, cap) in enumerate(((0, CAP0), (CUT1, CAP1),
                                        (CUT2, CAP2))):
            pos = np.nonzero(cl == ci)[0]
            pos = pos[np.argsort(t[pos], kind="stable")]
            n = pos.shape[0]
            if n > cap:
                raise ValueError(
                    f"core {b}: cluster {ci} has {n} tokens > capacity {cap}")
            gi = np.zeros(cap, dtype=np.int16)
            gi[:n] = (t[pos] - lo).astype(np.int16)
            di = np.full(cap, PAD_DEST, dtype=np.int32)
            di[:n] = pos.astype(np.int32)
            gs.append(gi)
            ds.append(di)
        maps.append({
            "head_tab": head, "t0_tab": t0b, "t1_tab": t1b,
            "w0T": w0T, "w1T": w1T,
            "g0": _wrap16(gs[0]), "g1": _wrap16(gs[1]), "g2": _wrap16(gs[2]),
            "d0": _slots(ds[0]), "d1": _slots(ds[1]), "d2": _slots(ds[2]),
        })
    return maps


def _ensure_axon_hooks():
    """bass_utils imports antenv.axon_hooks when tracing is requested via
    env; provide a no-op fallback module if the image lacks it."""
    import sys
    import types

    try:
        import antenv.axon_hooks  # noqa: F401
    except Exception:
        mod = types.ModuleType("antenv.axon_hooks")
        mod._hook = None
        mod.set_axon_ntff_profile_hook = lambda h: setattr(mod, "_hook", h)
        mod.get_axon_ntff_profile_hook = lambda: mod._hook
        sys.modules["antenv.axon_hooks"] = mod
        try:
            import antenv

            antenv.axon_hooks = mod
        except Exception:
            pass


def kernel(tokens, head_emb, tail0_emb, tail0_w, tail1_emb, tail1_w):
    _ensure_axon_hooks()
    from concourse.bass_utils import run_bass_kernel_spmd

    B, S = tokens.shape
    nc = _get_graph()
    in_maps = make_in_maps(tokens, head_emb, tail0_emb, tail0_w,
                           tail1_emb, tail1_w)
    res = run_bass_kernel_spmd(nc, in_maps, core_ids=list(range(B)))
    out = np.stack([r["out"] for r in res.results], axis=0)
    return out.reshape(B, S, D).astype(np.float32)
